# revision 1
# baseline (speedup 1.0000x reference)
"""Trainium2 Bass kernel for nn_ClassAwareLoss (class-aware frame loss).

Contract: kernel(**inputs) takes the FULL unsharded inputs (numpy arrays,
keyed as in setup_inputs()) and returns the FULL output (a float32 scalar).

Strategy (data-parallel over batch, per the sharding hint):
  - Shard `input`/`target` row-wise across 8 NeuronCores (2048 samples each).
  - Replicate the small tensors (frames^T, per-frame class ids, per-frame
    cosine weights) to every core.
  - Each core computes partial sums of
        caloss_c = sum_b sum_f [class(f)==t_b] * cosine_c[t_b] * (1 - d_bf)^2
        reg_c    = sum_b (||x_b|| - 1)^2
    and the host combines: (sum caloss + 6e-4 * sum reg) / B.

Device algorithm (per core, 2048 samples):
  dots are computed in bf16 on the PE (fp32 accumulate in PSUM); the
  normalization 1/||x|| is folded into the ScalarE pass that computes
  S = (1 - g*r)^2 via activation(Square, scale=-g, bias=1).  The
  class mask and per-frame cosine weight fuse into one DVE
  scalar_tensor_tensor op: w = (frame_class == t) * cosine_c[frame_class],
  and a tensor_tensor_reduce accumulates sum(w * S) per partition.
"""

import sys
import types
from contextlib import ExitStack

sys.path.insert(0, "/opt/trn_rl_repo")

import numpy as np
import ml_dtypes

# ---------------------------------------------------------------------------
# antenv.axon_hooks shim: lets run_bass_kernel_spmd(trace=True) capture NTFF
# profiles under axon.  Harmless when BASS_TRACE is not set.
# ---------------------------------------------------------------------------
try:
    import antenv

    if "antenv.axon_hooks" not in sys.modules:
        _mod = types.ModuleType("antenv.axon_hooks")
        _hook = [None]
        _mod.set_axon_ntff_profile_hook = lambda h: _hook.__setitem__(0, h)
        _mod.get_axon_ntff_profile_hook = lambda: _hook[0]
        sys.modules["antenv.axon_hooks"] = _mod
        antenv.axon_hooks = _mod
        try:
            from trn_agent_boot.trn_boot import _ntff_profile_via_ctypes

            _mod.set_axon_ntff_profile_hook(
                _ntff_profile_via_ctypes("/opt/axon/libaxon_pjrt.so")
            )
        except Exception:
            pass
except Exception:
    pass

import concourse.bass as bass
import concourse.tile as tile
import concourse.bass_utils as bass_utils
from concourse import bacc, mybir

# No cloud bucket in this container; keep artifacts local.
bass_utils.upload_artifacts = lambda tmpdir: "local://" + tmpdir

# ---------------------------------------------------------------------------
# Problem constants (from the reference problem definition; input-independent)
# ---------------------------------------------------------------------------
N_CORES = 8
B = 16384
D = 256
NCLS = 100
F_PARAM = 17
BS = B // N_CORES            # 2048 samples per core
NT = BS // 128               # 16 sample-tiles of 128 per core
F_TOTAL = NCLS * (F_PARAM - 1)  # 1600 frame rows

_CLS_SAMPLES = [5000 - 50 * i for i in range(100)]


def _calc_cls_idx(cls_samples, f):
    nc_ = len(cls_samples)
    n_samples = sum(cls_samples)
    ca_frame_num = [int((f - 2) * nc_ * r / n_samples) + 1 for r in cls_samples]
    over_flow = nc_ * (f - 1) - sum(ca_frame_num)
    for i in range(over_flow):
        ca_frame_num[i] += 1
    ca_frame_num.reverse()
    cls_frame_idx = [sum(ca_frame_num[0:k]) for k in range(nc_ + 1)]
    return cls_frame_idx, ca_frame_num


CLS_FRAME_IDX, CA_FRAME_NUM = _calc_cls_idx(_CLS_SAMPLES, F_PARAM)
FRAME_CLASS = np.repeat(np.arange(NCLS), CA_FRAME_NUM)  # [1600], deterministic

BF16 = mybir.dt.bfloat16
F32 = mybir.dt.float32
AF = mybir.ActivationFunctionType
ALU = mybir.AluOpType

_COMPILED = None   # (nc, meta)
LAST_RESULT = None  # BassKernelResults of the most recent run (for test.py)


def _build_program():
    """Build + compile the SPMD Bass program (one program, run on 8 cores)."""
    nc = bacc.Bacc(
        "TRN2", target_bir_lowering=False, debug=False, num_devices=N_CORES
    )

    # Per-core inputs
    x_bf = nc.dram_tensor("x_bf", [BS, D], BF16, kind="ExternalInput").ap()
    t_f32 = nc.dram_tensor("t_f32", [128, NT], F32, kind="ExternalInput").ap()
    framesT = nc.dram_tensor("framesT", [D, F_TOTAL], BF16, kind="ExternalInput").ap()
    iota_in = nc.dram_tensor("iota_mat", [128, 128], BF16, kind="ExternalInput").ap()
    cos_in = nc.dram_tensor("cosine_mat", [128, 128], BF16, kind="ExternalInput").ap()
    ct_in = nc.dram_tensor("ct_mat", [128, F_TOTAL], BF16, kind="ExternalInput").ap()
    out = nc.dram_tensor("out", [128, 2], F32, kind="ExternalOutput").ap()

    with tile.TileContext(nc) as tc:
        with ExitStack() as ctx:
            const_pool = ctx.enter_context(tc.tile_pool(name="const", bufs=1))
            work_pool = ctx.enter_context(tc.tile_pool(name="work", bufs=1))
            s_pool = ctx.enter_context(tc.tile_pool(name="s", bufs=3))
            w_pool = ctx.enter_context(tc.tile_pool(name="w", bufs=3))
            psum_pool = ctx.enter_context(
                tc.tile_pool(name="psum", bufs=2, space="PSUM")
            )
            psum_g = ctx.enter_context(
                tc.tile_pool(name="psumg", bufs=1, space="PSUM")
            )

            # ---- x transposed first: the dots matmuls gate everything ----
            xt0 = work_pool.tile([128, BS], BF16, tag="xt0")
            xt1 = work_pool.tile([128, BS], BF16, tag="xt1")
            nc.sync.dma_start_transpose(xt0[:], x_bf[:, 0:128])
            nc.scalar.dma_start_transpose(xt1[:], x_bf[:, 128:256])

            # ---- x natural layout [128, NT*D] (tile i at cols i*D..) ----
            xn = work_pool.tile([128, NT * D], BF16, tag="xn")
            nc.sync.dma_start(
                xn[:].rearrange("p (i d) -> p i d", i=NT),
                x_bf.rearrange("(i p) d -> p i d", p=128),
            )

            framesT_sb = const_pool.tile([128, 2 * F_TOTAL], BF16, tag="framesT")
            nc.sync.dma_start(framesT_sb[:, 0:F_TOTAL], framesT[0:128, :])
            nc.sync.dma_start(framesT_sb[:, F_TOTAL : 2 * F_TOTAL], framesT[128:256, :])
            iota_sb = const_pool.tile([128, 128], BF16, tag="iota")
            nc.sync.dma_start(iota_sb[:], iota_in[:])
            cos_sb = const_pool.tile([128, 128], BF16, tag="cos")
            nc.sync.dma_start(cos_sb[:], cos_in[:])
            t_sb = const_pool.tile([128, NT], F32, tag="t")
            nc.sync.dma_start(t_sb[:], t_f32[:])
            ct_sb = const_pool.tile([128, F_TOTAL], BF16, tag="ct")
            nc.sync.dma_start(ct_sb[:], ct_in[:])

            neg_one = const_pool.tile([128, 1], F32, tag="negone")
            nc.vector.memset(neg_one[:], -1.0)

            # ---- per-sample squared norms -> [128, NT] ----
            sq = work_pool.tile([128, NT], F32, tag="sq")
            sq_dump = work_pool.tile([128, D], F32, tag="sqd")
            for i in range(NT):
                nc.scalar.activation(
                    sq_dump[:],
                    xn[:, i * D : (i + 1) * D],
                    AF.Square,
                    accum_out=sq[:, i : i + 1],
                )
            # norm, 1/norm, (norm-1)^2
            norm = work_pool.tile([128, NT], F32, tag="norm")
            nc.scalar.activation(norm[:], sq[:], AF.Sqrt)
            g = work_pool.tile([128, NT], F32, tag="g")
            nc.vector.reciprocal(g[:], norm[:])
            regsq = work_pool.tile([128, NT], F32, tag="regsq")
            nc.scalar.activation(
                regsq[:], norm[:], AF.Square, bias=neg_one[:], scale=1.0
            )
            reg_col = work_pool.tile([128, 1], F32, tag="regcol")
            nc.vector.tensor_reduce(
                out=reg_col[:], in_=regsq[:], axis=mybir.AxisListType.X, op=ALU.add
            )

            # ---- main loop over sample tiles ----
            # caloss = sum_c sum_f CT[c,f] * G[c,f],
            # G[c,f] = sum_b cosine_c[t_b] * [t_b == c] * S[b,f]   (PE matmuls)
            g_ps = psum_g.tile([128, F_TOTAL], F32, tag="G")
            HALVES = [(0, 1024), (1024, F_TOTAL)]
            for i in range(NT):
                # ct_col = cosine_c[t_b]; P = ct_col * onehot(t_b)
                ct_dump = w_pool.tile([128, 128], BF16, tag="ctdump")
                ct_col = w_pool.tile([128, 1], F32, tag="ctcol")
                nc.vector.scalar_tensor_tensor(
                    out=ct_dump[:], in0=iota_sb[:], scalar=t_sb[:, i : i + 1],
                    in1=cos_sb[:], op0=ALU.is_equal, op1=ALU.mult,
                    accum_out=ct_col[:],
                )
                p_tile = w_pool.tile([128, 128], BF16, tag="p")
                nc.vector.tensor_scalar(
                    out=p_tile[:], in0=iota_sb[:],
                    scalar1=t_sb[:, i : i + 1], scalar2=ct_col[:],
                    op0=ALU.is_equal, op1=ALU.mult,
                )
                for (flo, fhi) in HALVES:
                    hw_ = fhi - flo
                    dots = psum_pool.tile([128, hw_], F32, tag="dots")
                    for c0 in range(flo, fhi, 512):
                        c1 = min(c0 + 512, fhi)
                        nc.tensor.matmul(
                            dots[:, c0 - flo : c1 - flo],
                            lhsT=xt0[:, i * 128 : (i + 1) * 128],
                            rhs=framesT_sb[:, c0:c1],
                            start=True,
                            stop=False,
                        )
                    for c0 in range(flo, fhi, 512):
                        c1 = min(c0 + 512, fhi)
                        nc.tensor.matmul(
                            dots[:, c0 - flo : c1 - flo],
                            lhsT=xt1[:, i * 128 : (i + 1) * 128],
                            rhs=framesT_sb[:, F_TOTAL + c0 : F_TOTAL + c1],
                            start=False,
                            stop=True,
                        )
                    # S = (g*r - 1)^2  (ScalarE: PSUM -> SBUF bf16)
                    s_tile = s_pool.tile([128, hw_], BF16, tag="s")
                    nc.scalar.activation(
                        s_tile[:], dots[:], AF.Square,
                        bias=neg_one[:], scale=g[:, i : i + 1],
                    )
                    # G[:, chunk] += P^T @ S
                    for c0 in range(flo, fhi, 512):
                        c1 = min(c0 + 512, fhi)
                        nc.tensor.matmul(
                            g_ps[:, c0:c1],
                            lhsT=p_tile[:],
                            rhs=s_tile[:, c0 - flo : c1 - flo],
                            start=(i == 0),
                            stop=(i == NT - 1),
                            skip_group_check=True,
                        )

            # total caloss per class-partition: sum_f CT * G
            g_dump = w_pool.tile([128, F_TOTAL], BF16, tag="gdump")
            cal_col = work_pool.tile([128, 1], F32, tag="calcol")
            nc.vector.scalar_tensor_tensor(
                out=g_dump[:], in0=ct_sb[:], scalar=1.0, in1=g_ps[:],
                op0=ALU.mult, op1=ALU.mult, accum_out=cal_col[:],
            )
            res_sb = work_pool.tile([128, 2], F32, tag="res")
            nc.vector.tensor_copy(res_sb[:, 0:1], cal_col[:])
            nc.vector.tensor_copy(res_sb[:, 1:2], reg_col[:])
            nc.sync.dma_start(out[:], res_sb[:])

    nc.compile()
    return nc


def _prepare_inputs(inputs):
    x = np.asarray(inputs["input"], dtype=np.float32)        # [B, D]
    frames = np.asarray(inputs["frames"], dtype=np.float32)  # [F, D]
    cosine_c = np.asarray(inputs["cosine_c"], dtype=np.float32)  # [NCLS]
    target = np.asarray(inputs["target"])                    # [B] int

    x_bf = x.astype(ml_dtypes.bfloat16)
    framesT = np.ascontiguousarray(frames.T).astype(ml_dtypes.bfloat16)  # [D, F]
    iota_mat = np.ascontiguousarray(
        np.broadcast_to(
            np.arange(128, dtype=np.float32).astype(ml_dtypes.bfloat16), (128, 128)
        )
    )
    cos_pad = np.zeros(128, np.float32)
    cos_pad[:NCLS] = cosine_c
    cosine_mat = np.ascontiguousarray(
        np.broadcast_to(cos_pad.astype(ml_dtypes.bfloat16), (128, 128))
    )
    ct_mat = np.zeros((128, F_TOTAL), np.float32)
    ct_mat[FRAME_CLASS, np.arange(F_TOTAL)] = 1.0
    ct_mat = ct_mat.astype(ml_dtypes.bfloat16)

    in_maps = []
    for c in range(N_CORES):
        sl = slice(c * BS, (c + 1) * BS)
        tc_ = target[sl].astype(np.float32).reshape(NT, 128).T
        # negate target? no: t values compared with fc via is_equal.
        in_maps.append(
            {
                "x_bf": np.ascontiguousarray(x_bf[sl]),
                "t_f32": np.ascontiguousarray(tc_),
                "framesT": framesT,
                "iota_mat": iota_mat,
                "cosine_mat": cosine_mat,
                "ct_mat": ct_mat,
            }
        )
    return in_maps


def kernel(**inputs):
    global _COMPILED, LAST_RESULT
    if _COMPILED is None:
        _COMPILED = _build_program()
    nc = _COMPILED

    in_maps = _prepare_inputs(inputs)
    res = bass_utils.run_bass_kernel_spmd(
        nc, in_maps, core_ids=list(range(N_CORES))
    )
    LAST_RESULT = res

    caloss = 0.0
    reg = 0.0
    for c in range(N_CORES):
        o = res.results[c]["out"].astype(np.float64)
        caloss += o[:, 0].sum()
        reg += o[:, 1].sum()
    val = (caloss + 0.0006 * reg) / B
    return np.float32(val)



# revision 9
# speedup vs baseline: 1.9072x; 1.9072x over previous
"""Trainium2 Bass kernel for nn_ClassAwareLoss (class-aware frame loss).

Contract: kernel(**inputs) takes the FULL unsharded inputs (numpy arrays,
keyed as in setup_inputs()) and returns the FULL output (a float32 scalar).

Strategy: the loss only touches, for each sample b with target class t,
the frames of class t (<= 31 of the 1600 frame rows; frame_class is the
deterministic sorted-by-class layout).  So instead of computing all
B x 1600 dots (the v0 kernel; PE-bound at ~91us), the host sorts samples
by class and packs them into 128-sample tiles whose class frames fit in a
128-wide contiguous frame window.  Each tile then needs a single
128x256 @ 256x128 matmul -- a ~12x cut in PE columns.

All data-dependence (the permutation, the per-tile frame windows, the
per-(sample,frame) weights cosine_c[t]*[frame_class==t]) is folded into
host-prepared tensor *content*, so one static SPMD program serves all
8 cores:

  per core (T tiles):
    x_in  [T*128, 256] bf16  permuted/padded samples (pad rows = e0)
    fwin  [256, T*W]   bf16  per-tile frame-window slices of frames^T
    wmat  [128, T*W]   bf16  per-tile weights c_t * [frame_class == t]

  device:
    sq_b = sum_d x^2            (DVE: square + 3D-view row-reduce)
    norm=sqrt(sq), g=1/norm, reg_b=(norm-1)^2
    per tile: dots = x_tile @ fwin_tile   (PE, two 128-contraction passes)
              S = (g*dots - 1)^2          (ScalarE, scale=g fused)
    caloss_col = sum(wmat * S_all)        (single DVE tensor_tensor_reduce)

Pad sample rows are e0 (unit norm): reg contribution exactly 0, caloss
contribution 0 via wmat=0, and no inf/NaN from g.
"""

import sys
import types
from contextlib import ExitStack

sys.path.insert(0, "/opt/trn_rl_repo")

import numpy as np
import ml_dtypes

# ---------------------------------------------------------------------------
# antenv.axon_hooks shim: lets run_bass_kernel_spmd(trace=True) capture NTFF
# profiles under axon.  Harmless when BASS_TRACE is not set.
# ---------------------------------------------------------------------------
try:
    import antenv

    if "antenv.axon_hooks" not in sys.modules:
        _mod = types.ModuleType("antenv.axon_hooks")
        _hook = [None]
        _mod.set_axon_ntff_profile_hook = lambda h: _hook.__setitem__(0, h)
        _mod.get_axon_ntff_profile_hook = lambda: _hook[0]
        sys.modules["antenv.axon_hooks"] = _mod
        antenv.axon_hooks = _mod
        try:
            from trn_agent_boot.trn_boot import _ntff_profile_via_ctypes

            _mod.set_axon_ntff_profile_hook(
                _ntff_profile_via_ctypes("/opt/axon/libaxon_pjrt.so")
            )
        except Exception:
            pass
except Exception:
    pass

import concourse.bass as bass
import concourse.tile as tile
import concourse.bass_utils as bass_utils
from concourse import bacc, mybir

# No cloud bucket in this container; keep artifacts local.
bass_utils.upload_artifacts = lambda tmpdir: "local://" + tmpdir

# ---------------------------------------------------------------------------
# Problem constants (from the reference problem definition; input-independent)
# ---------------------------------------------------------------------------
N_CORES = 8
B = 16384
D = 256
NCLS = 100
F_PARAM = 17
F_TOTAL = NCLS * (F_PARAM - 1)  # 1600 frame rows
W = 128                          # per-tile frame-window width

_CLS_SAMPLES = [5000 - 50 * i for i in range(100)]


def _calc_cls_idx(cls_samples, f):
    nc_ = len(cls_samples)
    n_samples = sum(cls_samples)
    ca_frame_num = [int((f - 2) * nc_ * r / n_samples) + 1 for r in cls_samples]
    over_flow = nc_ * (f - 1) - sum(ca_frame_num)
    for i in range(over_flow):
        ca_frame_num[i] += 1
    ca_frame_num.reverse()
    cls_frame_idx = [sum(ca_frame_num[0:k]) for k in range(nc_ + 1)]
    return cls_frame_idx, ca_frame_num


CLS_FRAME_IDX, CA_FRAME_NUM = _calc_cls_idx(_CLS_SAMPLES, F_PARAM)
FRAME_CLASS = np.repeat(np.arange(NCLS), CA_FRAME_NUM)  # [1600], deterministic

BF16 = mybir.dt.bfloat16
F32 = mybir.dt.float32
AF = mybir.ActivationFunctionType
ALU = mybir.AluOpType

_COMPILED = {}      # T -> compiled Bacc program
LAST_RESULT = None  # BassKernelResults of the most recent run (for test.py)


def _build_program(T, use_ttr=False, use_3d_sq=True, chunked_dma=True):
    """Build + compile the SPMD Bass program for T 128-sample tiles/core."""
    nc = bacc.Bacc(
        "TRN2", target_bir_lowering=False, debug=False, num_devices=N_CORES
    )
    TW = T * W
    TD = T * D
    CH = 2 if chunked_dma else 1  # split the norm chain for pipelining
    TCH = (T + CH - 1) // CH      # tiles per chunk

    x_in = nc.dram_tensor("x_in", [T * 128, D], BF16, kind="ExternalInput").ap()
    fwin = nc.dram_tensor("fwin", [D, TW], BF16, kind="ExternalInput").ap()
    wmat = nc.dram_tensor("wmat", [128, TW], BF16, kind="ExternalInput").ap()
    out = nc.dram_tensor("out", [128, 2], F32, kind="ExternalOutput").ap()

    with tile.TileContext(nc) as tc:
        with ExitStack() as ctx:
            const_pool = ctx.enter_context(tc.tile_pool(name="const", bufs=1))
            work_pool = ctx.enter_context(tc.tile_pool(name="work", bufs=1))
            psum_pool = ctx.enter_context(
                tc.tile_pool(name="psum", bufs=4, space="PSUM")
            )

            # ---- natural-layout x first: the norm chain gates the S pass ----
            xn = work_pool.tile([128, TD], BF16, tag="xn")
            xn3 = xn[:].rearrange("p (i d) -> p i d", i=T)
            for ch in range(CH):
                i0, i1 = ch * TCH, min((ch + 1) * TCH, T)
                nc.sync.dma_start(
                    xn3[:, i0:i1, :],
                    x_in.rearrange("(i p) d -> p i d", p=128)[:, i0:i1, :],
                )

            # ---- transposed x + frame windows (feed the PE) ----
            xt0 = work_pool.tile([128, T * 128], BF16, tag="xt0")
            xt1 = work_pool.tile([128, T * 128], BF16, tag="xt1")
            nc.sync.dma_start_transpose(xt0[:], x_in[:, 0:128])
            nc.scalar.dma_start_transpose(xt1[:], x_in[:, 128:256])
            fw_sb = const_pool.tile([128, 2 * TW], BF16, tag="fwin")
            nc.sync.dma_start(fw_sb[:, 0:TW], fwin[0:128, :])
            nc.sync.dma_start(fw_sb[:, TW : 2 * TW], fwin[128:256, :])
            wm_sb = const_pool.tile([128, TW], BF16, tag="wmat")
            nc.sync.dma_start(wm_sb[:], wmat[:])

            neg_one = const_pool.tile([128, 1], F32, tag="negone")
            nc.vector.memset(neg_one[:], -1.0)

            # ---- norm chain, chunked: sq -> norm -> g, reg ----
            xsq = work_pool.tile([128, TD], BF16, tag="xsq")
            xsq3 = xsq[:].rearrange("p (i d) -> p i d", i=T)
            sq = work_pool.tile([128, T], F32, tag="sq")
            norm = work_pool.tile([128, T], F32, tag="norm")
            g = work_pool.tile([128, T], F32, tag="g")
            regsq = work_pool.tile([128, T], F32, tag="regsq")
            for ch in range(CH):
                i0, i1 = ch * TCH, min((ch + 1) * TCH, T)
                if use_3d_sq:
                    nc.vector.scalar_tensor_tensor(
                        out=xsq3[:, i0:i1, :], in0=xn3[:, i0:i1, :], scalar=1.0,
                        in1=xn3[:, i0:i1, :], op0=ALU.mult, op1=ALU.mult,
                    )
                    nc.vector.tensor_reduce(
                        out=sq[:, i0:i1], in_=xsq3[:, i0:i1, :],
                        axis=mybir.AxisListType.X, op=ALU.add,
                    )
                else:
                    for i in range(i0, i1):
                        nc.scalar.activation(
                            xsq[:, i * D : (i + 1) * D],
                            xn[:, i * D : (i + 1) * D],
                            AF.Square,
                            accum_out=sq[:, i : i + 1],
                        )
                nc.scalar.activation(norm[:, i0:i1], sq[:, i0:i1], AF.Sqrt)
                nc.vector.reciprocal(g[:, i0:i1], norm[:, i0:i1])
                nc.scalar.activation(
                    regsq[:, i0:i1], norm[:, i0:i1], AF.Square,
                    bias=neg_one[:], scale=1.0,
                )
            reg_col = work_pool.tile([128, 1], F32, tag="regcol")
            nc.vector.tensor_reduce(
                out=reg_col[:], in_=regsq[:], axis=mybir.AxisListType.X, op=ALU.add
            )

            # ---- per-tile dots + S; S_all consumed by one weighted reduce ----
            s_all = work_pool.tile([128, TW], BF16, tag="s_all")
            for i in range(T):
                dots = psum_pool.tile([128, W], F32, tag="dots")
                nc.tensor.matmul(
                    dots[:],
                    lhsT=xt0[:, i * 128 : (i + 1) * 128],
                    rhs=fw_sb[:, i * W : (i + 1) * W],
                    start=True,
                    stop=False,
                )
                nc.tensor.matmul(
                    dots[:],
                    lhsT=xt1[:, i * 128 : (i + 1) * 128],
                    rhs=fw_sb[:, TW + i * W : TW + (i + 1) * W],
                    start=False,
                    stop=True,
                )
                # S = (g*r - 1)^2  (ScalarE: PSUM -> SBUF bf16)
                nc.scalar.activation(
                    s_all[:, i * W : (i + 1) * W], dots[:], AF.Square,
                    bias=neg_one[:], scale=g[:, i : i + 1],
                )

            cal_dump = work_pool.tile([128, TW], BF16, tag="caldump")
            cal_col = work_pool.tile([128, 1], F32, tag="calcol")
            if use_ttr:
                nc.vector.tensor_tensor_reduce(
                    out=cal_dump[:], in0=wm_sb[:], in1=s_all[:],
                    scale=1.0, scalar=0.0, op0=ALU.mult, op1=ALU.add,
                    accum_out=cal_col[:],
                )
            else:
                nc.vector.scalar_tensor_tensor(
                    out=cal_dump[:], in0=wm_sb[:], scalar=1.0, in1=s_all[:],
                    op0=ALU.mult, op1=ALU.mult, accum_out=cal_col[:],
                )

            res_sb = work_pool.tile([128, 2], F32, tag="res")
            nc.vector.tensor_copy(res_sb[:, 0:1], cal_col[:])
            nc.vector.tensor_copy(res_sb[:, 1:2], reg_col[:])
            nc.sync.dma_start(out[:], res_sb[:])

    nc.compile()
    return nc


def _pack_tiles(target):
    """Sort samples by class; pack into <=128-sample tiles whose class
    frames fit a W-wide window.  Returns (tiles, wlos): per tile the sample
    indices and the frame-window start."""
    order = np.argsort(target, kind="stable")
    tiles, wlos = [], []
    cur, lo, hi = [], 0, 0
    for s in order:
        t = int(target[s])
        a, b = CLS_FRAME_IDX[t], CLS_FRAME_IDX[t + 1]
        if cur and (len(cur) == 128 or max(hi, b) - lo > W):
            tiles.append(cur)
            wlos.append(lo)
            cur, lo, hi = [], a, b
        elif not cur:
            lo, hi = a, b
        cur.append(s)
        hi = max(hi, b)
    if cur:
        tiles.append(cur)
        wlos.append(lo)
    return tiles, wlos


def _prepare_inputs(inputs):
    x = np.asarray(inputs["input"], dtype=np.float32)            # [B, D]
    frames = np.asarray(inputs["frames"], dtype=np.float32)      # [F, D]
    cosine_c = np.asarray(inputs["cosine_c"], dtype=np.float32)  # [NCLS]
    target = np.asarray(inputs["target"]).astype(np.int64)       # [B]

    tiles, wlos = _pack_tiles(target)
    ntiles = len(tiles)
    T = (ntiles + N_CORES - 1) // N_CORES
    total = T * N_CORES
    TW = T * W

    # sample-index matrix [total, 128], -1 = pad
    samp = np.full((total, 128), -1, dtype=np.int64)
    wlo_arr = np.zeros(total, dtype=np.int64)
    for i, (tl, wl) in enumerate(zip(tiles, wlos)):
        samp[i, : len(tl)] = tl
        wlo_arr[i] = wl

    # permuted/padded x (pad rows = e0 -> unit norm, zero reg)
    xg = x[np.clip(samp, 0, B - 1)]                    # [total, 128, D]
    pad = samp < 0
    xg[pad] = 0.0
    xg[pad, 0] = 1.0
    xg_bf = xg.astype(ml_dtypes.bfloat16)

    # per-tile frame windows of frames^T
    framesT = np.ascontiguousarray(frames.T).astype(ml_dtypes.bfloat16)
    fwin_all = np.zeros((total, D, W), dtype=ml_dtypes.bfloat16)
    for i, wl in enumerate(wlo_arr):
        wd = min(W, F_TOTAL - wl)
        fwin_all[i, :, :wd] = framesT[:, wl : wl + wd]

    # per-tile weights: wmat[p, j] = cosine_c[t_p] * [frame_class[wlo+j]==t_p]
    fc_pad = np.full(F_TOTAL + W, -1, dtype=np.int64)
    fc_pad[:F_TOTAL] = FRAME_CLASS
    fcw = fc_pad[wlo_arr[:, None] + np.arange(W)[None, :]]       # [total, W]
    tval = np.where(samp >= 0, target[np.clip(samp, 0, B - 1)], -2)
    cos_pad = np.zeros(NCLS, np.float32)
    cos_pad[:] = cosine_c
    cval = np.where(samp >= 0, cos_pad[np.clip(tval, 0, NCLS - 1)], 0.0)
    wmat_all = (
        (tval[:, :, None] == fcw[:, None, :]).astype(np.float32)
        * cval[:, :, None]
    ).astype(ml_dtypes.bfloat16)                                  # [total,128,W]

    in_maps = []
    for c in range(N_CORES):
        sl = slice(c * T, (c + 1) * T)
        in_maps.append(
            {
                "x_in": np.ascontiguousarray(
                    xg_bf[sl].reshape(T * 128, D)
                ),
                "fwin": np.ascontiguousarray(
                    fwin_all[sl].transpose(1, 0, 2).reshape(D, TW)
                ),
                "wmat": np.ascontiguousarray(
                    wmat_all[sl].transpose(1, 0, 2).reshape(128, TW)
                ),
            }
        )
    return in_maps, T


def kernel(**inputs):
    global LAST_RESULT
    in_maps, T = _prepare_inputs(inputs)
    if T not in _COMPILED:
        _COMPILED[T] = _build_program(T)
    nc = _COMPILED[T]

    try:
        res = bass_utils.run_bass_kernel_spmd(
            nc, in_maps, core_ids=list(range(N_CORES))
        )
    except Exception:
        # one retry: a previous crashed run can leave the device wedged
        res = bass_utils.run_bass_kernel_spmd(
            nc, in_maps, core_ids=list(range(N_CORES))
        )
    LAST_RESULT = res

    caloss = 0.0
    reg = 0.0
    for c in range(N_CORES):
        o = res.results[c]["out"].astype(np.float64)
        caloss += o[:, 0].sum()
        reg += o[:, 1].sum()
    val = (caloss + 0.0006 * reg) / B
    return np.float32(val)


# revision 10
# speedup vs baseline: 2.4406x; 1.2797x over previous
"""Trainium2 Bass kernel for nn_ClassAwareLoss (class-aware frame loss).

Contract: kernel(**inputs) takes the FULL unsharded inputs (numpy arrays,
keyed as in setup_inputs()) and returns the FULL output (a float32 scalar).

Strategy: the loss only touches, for each sample b with target class t,
the frames of class t (<= 31 of the 1600 frame rows; frame_class is the
deterministic sorted-by-class layout).  So instead of computing all
B x 1600 dots (PE-bound at ~91us), the host sorts samples by class and
packs them into 128-sample tiles whose class frames fit a 128-wide
contiguous frame window.  Each tile then needs one 128x256 @ 256x128
matmul -- a ~12x cut in PE columns.

All data-dependence (the permutation, the per-tile frame windows, the
per-(sample,frame) weights cosine_c[t]*[frame_class==t]) is folded into
host-prepared tensor *content*, so one static SPMD program serves all
8 cores.  Every device input is laid out host-side as a partition-major
[128, N] contiguous array (transposes included) so each load is one
big-descriptor DMA.

  per core (T tiles):
    xn_in  [128, T*256]  natural-layout samples (tile i at cols i*256..)
    xt0/xt1 [128, T*128] transposed sample halves (matmul lhsT)
    fw_in  [128, 2*T*W]  per-tile frame-window slices of frames^T
    wm_in  [128, T*W]    per-tile weights c_t * [frame_class == t]

  device:
    sq_b = sum_d x^2            (DVE: square + 3D-view row-reduce, chunked)
    norm=sqrt(sq), g=1/norm, reg_b=(norm-1)^2
    per tile: dots = x_tile @ fwin_tile   (PE, two 128-contraction passes)
              S = (g*dots - 1)^2          (ScalarE, scale=g fused)
    caloss_col = sum(wmat * S_all)        (DVE scalar_tensor_tensor, halves)

Pad sample rows are e0 (unit norm): reg contribution exactly 0, caloss
contribution 0 via wmat=0, and no inf/NaN from g.
"""

import sys
import types
from contextlib import ExitStack

sys.path.insert(0, "/opt/trn_rl_repo")

import numpy as np
import ml_dtypes

# ---------------------------------------------------------------------------
# antenv.axon_hooks shim: lets run_bass_kernel_spmd(trace=True) capture NTFF
# profiles under axon.  Harmless when BASS_TRACE is not set.
# ---------------------------------------------------------------------------
try:
    import antenv

    if "antenv.axon_hooks" not in sys.modules:
        _mod = types.ModuleType("antenv.axon_hooks")
        _hook = [None]
        _mod.set_axon_ntff_profile_hook = lambda h: _hook.__setitem__(0, h)
        _mod.get_axon_ntff_profile_hook = lambda: _hook[0]
        sys.modules["antenv.axon_hooks"] = _mod
        antenv.axon_hooks = _mod
        try:
            from trn_agent_boot.trn_boot import _ntff_profile_via_ctypes

            _mod.set_axon_ntff_profile_hook(
                _ntff_profile_via_ctypes("/opt/axon/libaxon_pjrt.so")
            )
        except Exception:
            pass
except Exception:
    pass

import concourse.bass as bass
import concourse.tile as tile
import concourse.bass_utils as bass_utils
from concourse import bacc, mybir

# No cloud bucket in this container; keep artifacts local.
bass_utils.upload_artifacts = lambda tmpdir: "local://" + tmpdir

# ---------------------------------------------------------------------------
# Problem constants (from the reference problem definition; input-independent)
# ---------------------------------------------------------------------------
N_CORES = 8
B = 16384
D = 256
NCLS = 100
F_PARAM = 17
F_TOTAL = NCLS * (F_PARAM - 1)  # 1600 frame rows
W = 128                          # per-tile frame-window width

_CLS_SAMPLES = [5000 - 50 * i for i in range(100)]


def _calc_cls_idx(cls_samples, f):
    nc_ = len(cls_samples)
    n_samples = sum(cls_samples)
    ca_frame_num = [int((f - 2) * nc_ * r / n_samples) + 1 for r in cls_samples]
    over_flow = nc_ * (f - 1) - sum(ca_frame_num)
    for i in range(over_flow):
        ca_frame_num[i] += 1
    ca_frame_num.reverse()
    cls_frame_idx = [sum(ca_frame_num[0:k]) for k in range(nc_ + 1)]
    return cls_frame_idx, ca_frame_num


CLS_FRAME_IDX, CA_FRAME_NUM = _calc_cls_idx(_CLS_SAMPLES, F_PARAM)
FRAME_CLASS = np.repeat(np.arange(NCLS), CA_FRAME_NUM)  # [1600], deterministic

BF16 = mybir.dt.bfloat16
F32 = mybir.dt.float32
AF = mybir.ActivationFunctionType
ALU = mybir.AluOpType

_COMPILED = {}      # T -> compiled Bacc program
LAST_RESULT = None  # BassKernelResults of the most recent run (for test.py)


def _build_program(T):
    """Build + compile the SPMD Bass program for T 128-sample tiles/core."""
    nc = bacc.Bacc(
        "TRN2", target_bir_lowering=False, debug=False, num_devices=N_CORES
    )
    TW = T * W
    TD = T * D
    CH = min(4, T)               # norm-chain chunks (pipelining)
    TCH = (T + CH - 1) // CH     # tiles per chunk

    xn_in = nc.dram_tensor("xn_in", [128, TD], BF16, kind="ExternalInput").ap()
    xt0_in = nc.dram_tensor("xt0_in", [128, T * 128], BF16, kind="ExternalInput").ap()
    xt1_in = nc.dram_tensor("xt1_in", [128, T * 128], BF16, kind="ExternalInput").ap()
    fw_in = nc.dram_tensor("fw_in", [128, 2 * TW], BF16, kind="ExternalInput").ap()
    wm_in = nc.dram_tensor("wm_in", [128, TW], BF16, kind="ExternalInput").ap()
    out = nc.dram_tensor("out", [128, 2], F32, kind="ExternalOutput").ap()

    with tile.TileContext(nc) as tc:
        with ExitStack() as ctx:
            const_pool = ctx.enter_context(tc.tile_pool(name="const", bufs=1))
            work_pool = ctx.enter_context(tc.tile_pool(name="work", bufs=1))
            psum_pool = ctx.enter_context(
                tc.tile_pool(name="psum", bufs=4, space="PSUM")
            )

            xn = work_pool.tile([128, TD], BF16, tag="xn")
            xn3 = xn[:].rearrange("p (i d) -> p i d", i=T)
            xt0 = work_pool.tile([128, T * 128], BF16, tag="xt0")
            xt1 = work_pool.tile([128, T * 128], BF16, tag="xt1")
            fw_sb = const_pool.tile([128, 2 * TW], BF16, tag="fwin")
            wm_sb = const_pool.tile([128, TW], BF16, tag="wmat")

            # DMA plan: norm-chain feed (xn chunks) + fw first half on sync;
            # matmul lhsT + fw second half + weights on scalar.
            bounds = [
                (ch * TCH * D, min((ch + 1) * TCH, T) * D) for ch in range(CH)
            ]
            nc.sync.dma_start(xn[:, bounds[0][0] : bounds[0][1]],
                              xn_in[:, bounds[0][0] : bounds[0][1]])
            nc.scalar.dma_start(xt0[:], xt0_in[:])
            nc.sync.dma_start(fw_sb[:, 0:TW], fw_in[:, 0:TW])
            nc.scalar.dma_start(fw_sb[:, TW : 2 * TW], fw_in[:, TW : 2 * TW])
            for ch in range(1, CH):
                nc.sync.dma_start(xn[:, bounds[ch][0] : bounds[ch][1]],
                                  xn_in[:, bounds[ch][0] : bounds[ch][1]])
            nc.scalar.dma_start(xt1[:], xt1_in[:])
            nc.scalar.dma_start(wm_sb[:], wm_in[:])

            neg_one = const_pool.tile([128, 1], F32, tag="negone")
            nc.vector.memset(neg_one[:], -1.0)

            # ---- norm chain, chunked: sq -> norm -> g, reg ----
            xsq = work_pool.tile([128, TD], BF16, tag="xsq")
            xsq3 = xsq[:].rearrange("p (i d) -> p i d", i=T)
            sq = work_pool.tile([128, T], F32, tag="sq")
            norm = work_pool.tile([128, T], F32, tag="norm")
            g = work_pool.tile([128, T], F32, tag="g")
            regsq = work_pool.tile([128, T], F32, tag="regsq")
            for ch in range(CH):
                i0, i1 = ch * TCH, min((ch + 1) * TCH, T)
                nc.vector.scalar_tensor_tensor(
                    out=xsq3[:, i0:i1, :], in0=xn3[:, i0:i1, :], scalar=1.0,
                    in1=xn3[:, i0:i1, :], op0=ALU.mult, op1=ALU.mult,
                )
                nc.vector.tensor_reduce(
                    out=sq[:, i0:i1], in_=xsq3[:, i0:i1, :],
                    axis=mybir.AxisListType.X, op=ALU.add,
                )
                nc.scalar.activation(norm[:, i0:i1], sq[:, i0:i1], AF.Sqrt)
                nc.vector.reciprocal(g[:, i0:i1], norm[:, i0:i1])
                nc.scalar.activation(
                    regsq[:, i0:i1], norm[:, i0:i1], AF.Square,
                    bias=neg_one[:], scale=1.0,
                )
            reg_col = work_pool.tile([128, 1], F32, tag="regcol")
            nc.vector.tensor_reduce(
                out=reg_col[:], in_=regsq[:], axis=mybir.AxisListType.X, op=ALU.add
            )

            # ---- per-tile dots + S; S_all consumed by halved weighted reduce
            s_all = work_pool.tile([128, TW], BF16, tag="s_all")
            cal_dump = work_pool.tile([128, TW], BF16, tag="caldump")
            cal_cols = work_pool.tile([128, 2], F32, tag="calcols")
            half = (T + 1) // 2
            for i in range(T):
                dots = psum_pool.tile([128, W], F32, tag="dots")
                nc.tensor.matmul(
                    dots[:],
                    lhsT=xt0[:, i * 128 : (i + 1) * 128],
                    rhs=fw_sb[:, i * W : (i + 1) * W],
                    start=True,
                    stop=False,
                )
                nc.tensor.matmul(
                    dots[:],
                    lhsT=xt1[:, i * 128 : (i + 1) * 128],
                    rhs=fw_sb[:, TW + i * W : TW + (i + 1) * W],
                    start=False,
                    stop=True,
                )
                # S = (g*r - 1)^2  (ScalarE: PSUM -> SBUF bf16)
                nc.scalar.activation(
                    s_all[:, i * W : (i + 1) * W], dots[:], AF.Square,
                    bias=neg_one[:], scale=g[:, i : i + 1],
                )
                if i == half - 1 or i == T - 1:
                    lo = 0 if i == half - 1 else half * W
                    hi = (i + 1) * W
                    nc.vector.scalar_tensor_tensor(
                        out=cal_dump[:, lo:hi], in0=wm_sb[:, lo:hi], scalar=1.0,
                        in1=s_all[:, lo:hi], op0=ALU.mult, op1=ALU.mult,
                        accum_out=cal_cols[:, (0 if lo == 0 else 1) : (1 if lo == 0 else 2)],
                    )

            cal_col = work_pool.tile([128, 1], F32, tag="calcol")
            nc.vector.tensor_reduce(
                out=cal_col[:], in_=cal_cols[:], axis=mybir.AxisListType.X,
                op=ALU.add,
            )
            res_sb = work_pool.tile([128, 2], F32, tag="res")
            nc.vector.tensor_copy(res_sb[:, 0:1], cal_col[:])
            nc.vector.tensor_copy(res_sb[:, 1:2], reg_col[:])
            nc.sync.dma_start(out[:], res_sb[:])

    nc.compile()
    return nc


def _pack_tiles(target):
    """Sort samples by class; pack into <=128-sample tiles whose class
    frames fit a W-wide window.  Returns (tiles, wlos): per tile the sample
    indices and the frame-window start."""
    order = np.argsort(target, kind="stable")
    tiles, wlos = [], []
    cur, lo, hi = [], 0, 0
    for s in order:
        t = int(target[s])
        a, b = CLS_FRAME_IDX[t], CLS_FRAME_IDX[t + 1]
        if cur and (len(cur) == 128 or max(hi, b) - lo > W):
            tiles.append(cur)
            wlos.append(lo)
            cur, lo, hi = [], a, b
        elif not cur:
            lo, hi = a, b
        cur.append(s)
        hi = max(hi, b)
    if cur:
        tiles.append(cur)
        wlos.append(lo)
    return tiles, wlos


def _prepare_inputs(inputs):
    x = np.asarray(inputs["input"], dtype=np.float32)            # [B, D]
    frames = np.asarray(inputs["frames"], dtype=np.float32)      # [F, D]
    cosine_c = np.asarray(inputs["cosine_c"], dtype=np.float32)  # [NCLS]
    target = np.asarray(inputs["target"]).astype(np.int64)       # [B]

    tiles, wlos = _pack_tiles(target)
    ntiles = len(tiles)
    T = (ntiles + N_CORES - 1) // N_CORES
    total = T * N_CORES
    TW = T * W

    # sample-index matrix [total, 128], -1 = pad
    samp = np.full((total, 128), -1, dtype=np.int64)
    wlo_arr = np.zeros(total, dtype=np.int64)
    for i, (tl, wl) in enumerate(zip(tiles, wlos)):
        samp[i, : len(tl)] = tl
        wlo_arr[i] = wl

    # permuted/padded x (pad rows = e0 -> unit norm, zero reg)
    xg = x[np.clip(samp, 0, B - 1)]                    # [total, 128, D]
    pad = samp < 0
    xg[pad] = 0.0
    xg[pad, 0] = 1.0
    xg_bf = xg.astype(ml_dtypes.bfloat16)

    # per-tile frame windows of frames^T
    framesT = np.ascontiguousarray(frames.T).astype(ml_dtypes.bfloat16)
    fwin_all = np.zeros((total, D, W), dtype=ml_dtypes.bfloat16)
    for i, wl in enumerate(wlo_arr):
        wd = min(W, F_TOTAL - wl)
        fwin_all[i, :, :wd] = framesT[:, wl : wl + wd]

    # per-tile weights: wmat[p, j] = cosine_c[t_p] * [frame_class[wlo+j]==t_p]
    fc_pad = np.full(F_TOTAL + W, -1, dtype=np.int64)
    fc_pad[:F_TOTAL] = FRAME_CLASS
    fcw = fc_pad[wlo_arr[:, None] + np.arange(W)[None, :]]       # [total, W]
    tval = np.where(samp >= 0, target[np.clip(samp, 0, B - 1)], -2)
    cval = np.where(samp >= 0, cosine_c[np.clip(tval, 0, NCLS - 1)], 0.0)
    wmat_all = (
        (tval[:, :, None] == fcw[:, None, :]).astype(np.float32)
        * cval[:, :, None]
    ).astype(ml_dtypes.bfloat16)                                  # [total,128,W]

    in_maps = []
    for c in range(N_CORES):
        sl = slice(c * T, (c + 1) * T)
        xp = xg_bf[sl].reshape(T * 128, D)            # [T*128, 256]
        in_maps.append(
            {
                "xn_in": np.ascontiguousarray(
                    xg_bf[sl].transpose(1, 0, 2).reshape(128, T * D)
                ),
                "xt0_in": np.ascontiguousarray(xp[:, 0:128].T),
                "xt1_in": np.ascontiguousarray(xp[:, 128:256].T),
                "fw_in": np.ascontiguousarray(
                    np.concatenate(
                        [
                            fwin_all[sl, 0:128].transpose(1, 0, 2).reshape(128, TW),
                            fwin_all[sl, 128:256].transpose(1, 0, 2).reshape(128, TW),
                        ],
                        axis=1,
                    )
                ),
                "wm_in": np.ascontiguousarray(
                    wmat_all[sl].transpose(1, 0, 2).reshape(128, TW)
                ),
            }
        )
    return in_maps, T


def kernel(**inputs):
    global LAST_RESULT
    in_maps, T = _prepare_inputs(inputs)
    if T not in _COMPILED:
        _COMPILED[T] = _build_program(T)
    nc = _COMPILED[T]

    try:
        res = bass_utils.run_bass_kernel_spmd(
            nc, in_maps, core_ids=list(range(N_CORES))
        )
    except Exception:
        # one retry: a previous crashed run can leave the device wedged
        res = bass_utils.run_bass_kernel_spmd(
            nc, in_maps, core_ids=list(range(N_CORES))
        )
    LAST_RESULT = res

    caloss = 0.0
    reg = 0.0
    for c in range(N_CORES):
        o = res.results[c]["out"].astype(np.float64)
        caloss += o[:, 0].sum()
        reg += o[:, 1].sum()
    val = (caloss + 0.0006 * reg) / B
    return np.float32(val)
